# revision 2
# baseline (speedup 1.0000x reference)
"""Trainium2 Bass kernel for masked pairwise-sigmoid GNN message passing.

Reference computation (per graph g with nodes i,j in [0,nv)):
    c = z @ Wc.T + bc ; y = z @ Wy.T + by          # [G, nv, H]
    s[g,i,j,:] = sigmoid(c[g,i,:] + y[g,j,:] + (m_i + m_j)*L - 2L)
    out[g,i,:] = sum_j s[g,i,j,:] / sum_j m[g,j]

Key exact identity: with m in {0,1}, any pair with m_i==0 or m_j==0 has
mask term <= -1e10, so sigmoid underflows to exactly 0 in fp32.  Hence
only "active" nodes (m==1) contribute, and for active pairs the mask
term is exactly 0.  The host gathers active nodes per graph, the device
computes the dense active x active interaction, and the host scatters
rows back (inactive rows are exactly 0).

Sharding: graphs are sorted by active count and dealt round-robin to the
8 cores in 4 "slots"; slot s is padded to a single global size P_s so
one SPMD program serves all cores.  Padding columns get a -1e5 additive
mask (sigmoid -> 0) and padding rows are discarded on scatter.
"""

import numpy as np

import concourse.bass as bass
import concourse.mybir as mybir
import concourse.tile as tile
from concourse import bacc
from concourse.bass_utils import run_bass_kernel_spmd
from concourse.masks import make_identity

F32 = mybir.dt.float32
N_CORES = 8
PAD_NEG = -1.0e5  # additive mask for padding columns; sigmoid(-1e5) == 0 in fp32

# test.py reads this for profiling info after a traced run
_last_results = None


def _ceil_to(x: int, m: int) -> int:
    return ((x + m - 1) // m) * m


def _build_program(P_list, H):
    """One-core program; SPMD-replicated over 8 cores with different data."""
    NTOT = sum(P_list)
    KB = H // 128  # k blocks (2 for H=256)
    OB = H // 128  # output blocks
    assert H % 128 == 0

    nc = bacc.Bacc(None, target_bir_lowering=False)

    zT = nc.dram_tensor("zT", [H, NTOT], F32, kind="ExternalInput")
    wcT = nc.dram_tensor("wcT", [H, H], F32, kind="ExternalInput")
    wyT = nc.dram_tensor("wyT", [H, H], F32, kind="ExternalInput")
    bcT = nc.dram_tensor("bcT", [128, OB], F32, kind="ExternalInput")
    byT = nc.dram_tensor("byT", [128, OB], F32, kind="ExternalInput")
    madd = nc.dram_tensor("madd", [1, NTOT], F32, kind="ExternalInput")
    recipN = nc.dram_tensor("recipN", [NTOT], F32, kind="ExternalInput")
    out = nc.dram_tensor("out", [NTOT, H], F32, kind="ExternalOutput")

    AT = mybir.ActivationFunctionType
    OP = mybir.AluOpType

    with tile.TileContext(nc) as tc:
        with (
            tc.tile_pool(name="singles", bufs=1) as singles,
            tc.tile_pool(name="work", bufs=2) as work,
            tc.tile_pool(name="outp", bufs=2) as outp,
            tc.tile_pool(name="psum", bufs=2, space="PSUM") as psum,
            tc.tile_pool(name="psumt", bufs=2, space="PSUM") as psumt,
        ):
            # ---- load constants / inputs ----
            w_sb = {}
            for name, dram in (("c", wcT), ("y", wyT)):
                for kb in range(KB):
                    t = singles.tile([128, H], F32, tag=f"w{name}{kb}", name=f"w{name}{kb}")
                    nc.sync.dma_start(out=t[:], in_=dram[kb * 128:(kb + 1) * 128, :])
                    w_sb[name, kb] = t
            z_sb = []
            for kb in range(KB):
                t = singles.tile([128, NTOT], F32, tag=f"z{kb}", name=f"z{kb}")
                nc.sync.dma_start(out=t[:], in_=zT[kb * 128:(kb + 1) * 128, :])
                z_sb.append(t)
            bc_sb = singles.tile([128, OB], F32, tag="bc", name="bc_sb")
            nc.sync.dma_start(out=bc_sb[:], in_=bcT[:])
            by_sb = singles.tile([128, OB], F32, tag="by", name="by_sb")
            nc.sync.dma_start(out=by_sb[:], in_=byT[:])
            # madd replicated over all 128 partitions via a step-0 DMA read
            madd_rep = singles.tile([128, NTOT], F32, tag="madd", name="madd_rep")
            madd_bcast = bass.AP(
                tensor=madd[:].tensor,
                offset=madd[:].offset,
                ap=[[0, 128], [1, NTOT]],
            )
            nc.sync.dma_start(out=madd_rep[:], in_=madd_bcast)
            recip_sb = []
            col = 0
            while col < NTOT:
                ch = min(128, NTOT - col)
                t = singles.tile([128, 1], F32, tag=f"recip{len(recip_sb)}", name=f"recip{len(recip_sb)}")
                nc.sync.dma_start(
                    out=t[:ch, :], in_=recipN[col:col + ch].unsqueeze(1)
                )
                recip_sb.append(t)
                col += ch
            ident = singles.tile([128, 128], F32, tag="ident", name="ident")
            make_identity(nc, ident[:])

            # ---- projections: C'[o, n] = (Wc @ z^T)[o, n] + bc[o] (+madd for y)
            ct_sb = [singles.tile([128, NTOT], F32, tag=f"ct{ob}", name=f"ct{ob}") for ob in range(OB)]
            yt_sb = [singles.tile([128, NTOT], F32, tag=f"yt{ob}", name=f"yt{ob}") for ob in range(OB)]
            for name, dest, bias_sb, with_madd in (
                ("c", ct_sb, bc_sb, False),
                ("y", yt_sb, by_sb, True),
            ):
                for ob in range(OB):
                    ps = psum.tile([128, NTOT], F32, name="ps")
                    for kb in range(KB):
                        nc.tensor.matmul(
                            ps[:],
                            lhsT=w_sb[name, kb][:, ob * 128:(ob + 1) * 128],
                            rhs=z_sb[kb][:],
                            start=(kb == 0),
                            stop=(kb == KB - 1),
                        )
                    if with_madd:
                        nc.vector.scalar_tensor_tensor(
                            out=dest[ob][:],
                            in0=ps[:],
                            scalar=bias_sb[:, ob:ob + 1],
                            in1=madd_rep[:],
                            op0=OP.add,
                            op1=OP.add,
                        )
                    else:
                        nc.vector.tensor_scalar_add(
                            out=dest[ob][:], in0=ps[:], scalar1=bias_sb[:, ob:ob + 1]
                        )

            # ---- main loop: per (slot, h-block) pairwise add -> sigmoid -> sum_j
            out_sb = [singles.tile([128, NTOT], F32, tag=f"o{ob}", name=f"osb{ob}") for ob in range(OB)]
            col = 0
            for P in P_list:
                for ob in range(OB):
                    cpart = ct_sb[ob][:, col:col + P]  # [128, P] (i)
                    ypart = yt_sb[ob][:, col:col + P]  # [128, P] (j)
                    # in0[p, i, j] = c'[p, i]; in1[p, i, j] = y'[p, j]
                    in0 = bass.AP(
                        tensor=cpart.tensor,
                        offset=cpart.offset,
                        ap=[list(cpart.ap[0]), list(cpart.ap[1]), [0, P]],
                    )
                    in1 = bass.AP(
                        tensor=ypart.tensor,
                        offset=ypart.offset,
                        ap=[list(ypart.ap[0]), [0, P], list(ypart.ap[1])],
                    )
                    pt = work.tile([128, P, P], F32, tag="pair", name="pair_t")
                    nc.vector.tensor_tensor(out=pt[:], in0=in0, in1=in1, op=OP.add)
                    st = work.tile([128, P, P], F32, tag="sig", name="sig_t")
                    nc.scalar.activation(out=st[:], in_=pt[:], func=AT.Sigmoid)
                    nc.vector.reduce_sum(
                        out=out_sb[ob][:, col:col + P],
                        in_=st[:],
                        axis=mybir.AxisListType.X,
                    )
                col += P

            # ---- transpose [o, n] -> [n, o], scale by 1/denom, store
            for ob in range(OB):
                col = 0
                ci = 0
                while col < NTOT:
                    ch = min(128, NTOT - col)
                    pt = psumt.tile([128, 128], F32, name="pt_t")
                    nc.tensor.transpose(
                        pt[:ch, :], out_sb[ob][:, col:col + ch], ident[:]
                    )
                    ot = outp.tile([128, 128], F32, name="ot_t")
                    nc.vector.tensor_scalar_mul(
                        ot[:ch, :], pt[:ch, :], recip_sb[ci][:ch, :]
                    )
                    nc.sync.dma_start(
                        out=out[col:col + ch, ob * 128:(ob + 1) * 128],
                        in_=ot[:ch, :],
                    )
                    col += ch
                    ci += 1

    nc.finalize()
    return nc


def kernel(num_graphs, nv, z, mask, Wc, bc, Wy, by):
    global _last_results
    G = int(num_graphs)
    NV = int(nv)
    z = np.ascontiguousarray(np.asarray(z, dtype=np.float32))
    mask = np.asarray(mask, dtype=np.float32).reshape(G, NV)
    Wc = np.asarray(Wc, dtype=np.float32)
    bc = np.asarray(bc, dtype=np.float32)
    Wy = np.asarray(Wy, dtype=np.float32)
    by = np.asarray(by, dtype=np.float32)
    H = z.shape[-1]
    zg = z.reshape(G, NV, H)

    out_full = np.zeros((G * NV, H), dtype=np.float32)

    # ---- host: active-node compaction & slot assignment ----
    act_idx = [np.nonzero(mask[g] > 0.5)[0] for g in range(G)]
    n_act = np.array([len(a) for a in act_idx])
    for g in range(G):
        if n_act[g] == 0:  # reference: 0/0 -> NaN for the whole graph
            out_full[g * NV:(g + 1) * NV, :] = np.nan

    order = np.argsort(-n_act, kind="stable")  # graphs by count, descending
    n_slots = (G + N_CORES - 1) // N_CORES
    # slot s on every core holds graphs ranked [s*N_CORES, (s+1)*N_CORES)
    assign = [[None] * n_slots for _ in range(N_CORES)]
    P_list = []
    for s in range(n_slots):
        ranks = order[s * N_CORES:(s + 1) * N_CORES]
        for c, g in enumerate(ranks):
            assign[c][s] = int(g)
        mx = max((int(n_act[g]) for g in ranks), default=0)
        P_list.append(max(4, _ceil_to(mx, 4)))
    NTOT = sum(P_list)
    offs = np.cumsum([0] + P_list[:-1])

    # ---- host: per-core input staging ----
    wcT = np.ascontiguousarray(Wc.T)  # [h_in, o]
    wyT = np.ascontiguousarray(Wy.T)
    OB = H // 128
    bcT = np.ascontiguousarray(bc.reshape(OB, 128).T)  # [128, OB]
    byT = np.ascontiguousarray(by.reshape(OB, 128).T)

    in_maps = []
    for c in range(N_CORES):
        zT_act = np.zeros((H, NTOT), dtype=np.float32)
        madd = np.full((1, NTOT), PAD_NEG, dtype=np.float32)
        recipN = np.zeros((NTOT,), dtype=np.float32)
        for s in range(n_slots):
            g = assign[c][s]
            if g is None:
                continue
            n = int(n_act[g])
            if n == 0:
                continue
            o = int(offs[s])
            zT_act[:, o:o + n] = zg[g][act_idx[g]].T
            madd[0, o:o + n] = 0.0
            recipN[o:o + n] = np.float32(1.0) / np.float32(n)
        in_maps.append(
            {
                "zT": zT_act,
                "wcT": wcT,
                "wyT": wyT,
                "bcT": bcT,
                "byT": byT,
                "madd": madd,
                "recipN": recipN,
            }
        )

    # ---- build + run ----
    nc = _build_program(P_list, H)
    res = run_bass_kernel_spmd(nc, in_maps, list(range(N_CORES)))
    _last_results = res

    # ---- host: scatter back ----
    for c in range(N_CORES):
        oc = res.results[c]["out"]  # [NTOT, H]
        for s in range(n_slots):
            g = assign[c][s]
            if g is None:
                continue
            n = int(n_act[g])
            if n == 0:
                continue
            o = int(offs[s])
            out_full[g * NV + act_idx[g], :] = oc[o:o + n, :]

    return out_full


# revision 6
# speedup vs baseline: 1.0297x; 1.0297x over previous
"""Trainium2 Bass kernel for masked pairwise-sigmoid GNN message passing.

Reference computation (per graph g with nodes i,j in [0,nv)):
    c = z @ Wc.T + bc ; y = z @ Wy.T + by          # [G, nv, H]
    s[g,i,j,:] = sigmoid(c[g,i,:] + y[g,j,:] + (m_i + m_j)*L - 2L)
    out[g,i,:] = sum_j s[g,i,j,:] / sum_j m[g,j]

Key exact identity: with m in {0,1}, any pair with m_i==0 or m_j==0 has
mask term <= -1e10, so sigmoid underflows to exactly 0 in fp32.  Hence
only "active" nodes (m==1) contribute, and for active pairs the mask
term is exactly 0.  The host gathers active nodes per graph, the device
computes the dense active x active interaction, and the host scatters
rows back (inactive rows are exactly 0).

Sharding: graphs are sorted by active count and dealt round-robin to the
8 cores in 4 "slots"; slot s is padded to a single global size P_s so
one SPMD program serves all cores.  Padding columns get a -1e5 additive
mask (sigmoid -> 0) and padding rows are discarded on scatter.

Device pipeline per (slot, h-block):
  DVE: pairwise add via broadcast APs -> ACT: sigmoid (bf16 out)
  -> DVE: reduce over j (bf16) -> PE: transpose -> DVE: scale(1/denom)
  -> DMA out.  Projections run on PE; input DMAs are spread over
  several engines' HWDGE queues for parallelism.
"""

import numpy as np

import concourse.bass as bass
import concourse.mybir as mybir
import concourse.tile as tile
from concourse import bacc
from concourse.bass_utils import run_bass_kernel_spmd
from concourse.masks import make_identity

F32 = mybir.dt.float32
BF16 = mybir.dt.bfloat16
N_CORES = 8
PAD_NEG = -1.0e5  # additive mask for padding columns; sigmoid(-1e5) == 0

# test.py reads this for profiling info after a traced run
_last_results = None


def _ceil_to(x: int, m: int) -> int:
    return ((x + m - 1) // m) * m


def _build_program(P_list, H, sig_bf16=True):
    """One-core program; SPMD-replicated over 8 cores with different data."""
    NTOT = sum(P_list)
    KB = H // 128  # contraction blocks
    OB = H // 128  # output h blocks
    assert H % 128 == 0

    nc = bacc.Bacc(None, target_bir_lowering=False)

    zT = nc.dram_tensor("zT", [H, NTOT], F32, kind="ExternalInput")
    wcT = nc.dram_tensor("wcT", [H, H], F32, kind="ExternalInput")
    wyT = nc.dram_tensor("wyT", [H, H], F32, kind="ExternalInput")
    bcT = nc.dram_tensor("bcT", [128, OB], F32, kind="ExternalInput")
    byT = nc.dram_tensor("byT", [128, OB], F32, kind="ExternalInput")
    madd = nc.dram_tensor("madd", [1, NTOT], F32, kind="ExternalInput")
    recipN = nc.dram_tensor("recipN", [len(P_list), 128], F32, kind="ExternalInput")
    out = nc.dram_tensor("out", [NTOT, H], F32, kind="ExternalOutput")

    AT = mybir.ActivationFunctionType
    OP = mybir.AluOpType
    SDT = BF16 if sig_bf16 else F32

    with tile.TileContext(nc) as tc:
        with (
            tc.tile_pool(name="singles", bufs=1) as singles,
            tc.tile_pool(name="work", bufs=2) as work,
            tc.tile_pool(name="outp", bufs=2) as outp,
            tc.tile_pool(name="psum", bufs=2, space="PSUM") as psum,
            tc.tile_pool(name="psumt", bufs=2, space="PSUM") as psumt,
        ):
            # ---- load inputs; spread big loads over distinct engine queues
            z_sb = []
            for kb in range(KB):
                t = singles.tile([128, NTOT], F32, tag=f"z{kb}", name=f"z{kb}")
                nc.gpsimd.dma_start(out=t[:], in_=zT[kb * 128:(kb + 1) * 128, :])
                z_sb.append(t)
            w_sb = {}
            w_engines = {"c": nc.sync, "y": nc.scalar}
            for wname, dram in (("c", wcT), ("y", wyT)):
                for kb in range(KB):
                    t = singles.tile(
                        [128, H], F32, tag=f"w{wname}{kb}", name=f"w{wname}{kb}"
                    )
                    # split the [128, H] load into per-o-block halves so the
                    # first projection's weights land sooner
                    eng = w_engines[wname]
                    for ob in range(OB):
                        eng.dma_start(
                            out=t[:, ob * 128:(ob + 1) * 128],
                            in_=dram[kb * 128:(kb + 1) * 128,
                                     ob * 128:(ob + 1) * 128],
                        )
                    w_sb[wname, kb] = t
            bc_sb = singles.tile([128, OB], F32, tag="bc", name="bc_sb")
            nc.gpsimd.dma_start(out=bc_sb[:], in_=bcT[:])
            by_sb = singles.tile([128, OB], F32, tag="by", name="by_sb")
            nc.gpsimd.dma_start(out=by_sb[:], in_=byT[:])
            # madd replicated over all 128 partitions via a step-0 DMA read
            madd_rep = singles.tile([128, NTOT], F32, tag="madd", name="madd_rep")
            madd_bcast = bass.AP(
                tensor=madd[:].tensor,
                offset=madd[:].offset,
                ap=[[0, 128], [1, NTOT]],
            )
            nc.gpsimd.dma_start(out=madd_rep[:], in_=madd_bcast)
            recip_sb = []
            for s in range(len(P_list)):
                t = singles.tile(
                    [128, 1], F32, tag=f"recip{s}", name=f"recip{s}",
                )
                nc.gpsimd.dma_start(out=t[:], in_=recipN[s, :].unsqueeze(1))
                recip_sb.append(t)
            ident = singles.tile([128, 128], SDT, tag="ident", name="ident")
            make_identity(nc, ident[:])

            # ---- projections: C'[o, n] = (Wc @ z^T)[o, n] + bc[o] (+madd for y)
            ct_sb = [
                singles.tile([128, NTOT], F32, tag=f"ct{ob}", name=f"ct{ob}")
                for ob in range(OB)
            ]
            yt_sb = [
                singles.tile([128, NTOT], F32, tag=f"yt{ob}", name=f"yt{ob}")
                for ob in range(OB)
            ]
            # order (c,ob0), (y,ob0), (c,ob1), (y,ob1): the first slot's
            # pairwise add needs ob0 of both projections as early as possible
            proj_jobs = []
            for ob in range(OB):
                proj_jobs.append(("c", ob, ct_sb, bc_sb, False))
                proj_jobs.append(("y", ob, yt_sb, by_sb, True))
            for wname, ob, dest, bias_sb, with_madd in proj_jobs:
                ps = psum.tile([128, NTOT], F32, name="ps")
                for kb in range(KB):
                    nc.tensor.matmul(
                        ps[:],
                        lhsT=w_sb[wname, kb][:, ob * 128:(ob + 1) * 128],
                        rhs=z_sb[kb][:],
                        start=(kb == 0),
                        stop=(kb == KB - 1),
                    )
                if with_madd:
                    nc.vector.scalar_tensor_tensor(
                        out=dest[ob][:],
                        in0=ps[:],
                        scalar=bias_sb[:, ob:ob + 1],
                        in1=madd_rep[:],
                        op0=OP.add,
                        op1=OP.add,
                    )
                else:
                    nc.vector.tensor_scalar_add(
                        out=dest[ob][:], in0=ps[:], scalar1=bias_sb[:, ob:ob + 1]
                    )

            # ---- main loop + per-slot transpose/scale/store
            out_sb = [
                singles.tile([128, NTOT], SDT, tag=f"o{ob}", name=f"osb{ob}")
                for ob in range(OB)
            ]
            out_dma_engines = [nc.sync, nc.scalar]
            col = 0
            for si, P in enumerate(P_list):
                for ob in range(OB):
                    cpart = ct_sb[ob][:, col:col + P]  # [128, P] (i)
                    ypart = yt_sb[ob][:, col:col + P]  # [128, P] (j)
                    # in0[p, i, j] = c'[p, i]; in1[p, i, j] = y'[p, j]
                    in0 = bass.AP(
                        tensor=cpart.tensor,
                        offset=cpart.offset,
                        ap=[list(cpart.ap[0]), list(cpart.ap[1]), [0, P]],
                    )
                    in1 = bass.AP(
                        tensor=ypart.tensor,
                        offset=ypart.offset,
                        ap=[list(ypart.ap[0]), [0, P], list(ypart.ap[1])],
                    )
                    pt = work.tile([128, P, P], F32, tag="pair", name="pair_t")
                    nc.vector.tensor_tensor(out=pt[:], in0=in0, in1=in1, op=OP.add)
                    st = work.tile([128, P, P], SDT, tag="sig", name="sig_t")
                    nc.scalar.activation(out=st[:], in_=pt[:], func=AT.Sigmoid)
                    with nc.allow_low_precision(
                        "sum of <=128 sigmoid values in [0,1]; bf16 out is fine"
                    ):
                        nc.vector.reduce_sum(
                            out=out_sb[ob][:, col:col + P],
                            in_=st[:],
                            axis=mybir.AxisListType.X,
                        )
                # transpose this slot's [128, P] -> [P, 128] per h-block,
                # scale rows by 1/denom, store
                for ob in range(OB):
                    ptp = psumt.tile([128, 128], SDT, name="pt_t")
                    nc.tensor.transpose(
                        ptp[:P, :], out_sb[ob][:, col:col + P], ident[:]
                    )
                    ot = outp.tile([128, 128], F32, name="ot_t")
                    nc.vector.tensor_scalar_mul(
                        ot[:P, :], ptp[:P, :], recip_sb[si][:P, :]
                    )
                    out_dma_engines[ob].dma_start(
                        out=out[col:col + P, ob * 128:(ob + 1) * 128],
                        in_=ot[:P, :],
                    )
                col += P

    nc.finalize()
    return nc


def kernel(num_graphs, nv, z, mask, Wc, bc, Wy, by):
    global _last_results
    G = int(num_graphs)
    NV = int(nv)
    z = np.ascontiguousarray(np.asarray(z, dtype=np.float32))
    mask = np.asarray(mask, dtype=np.float32).reshape(G, NV)
    Wc = np.asarray(Wc, dtype=np.float32)
    bc = np.asarray(bc, dtype=np.float32)
    Wy = np.asarray(Wy, dtype=np.float32)
    by = np.asarray(by, dtype=np.float32)
    H = z.shape[-1]
    zg = z.reshape(G, NV, H)

    out_full = np.zeros((G * NV, H), dtype=np.float32)

    # ---- host: active-node compaction & slot assignment ----
    act_idx = [np.nonzero(mask[g] > 0.5)[0] for g in range(G)]
    n_act = np.array([len(a) for a in act_idx])
    for g in range(G):
        if n_act[g] == 0:  # reference: 0/0 -> NaN for the whole graph
            out_full[g * NV:(g + 1) * NV, :] = np.nan

    order = np.argsort(-n_act, kind="stable")  # graphs by count, descending
    n_slots = (G + N_CORES - 1) // N_CORES
    assign = [[None] * n_slots for _ in range(N_CORES)]
    P_list = []
    for s in range(n_slots):
        ranks = order[s * N_CORES:(s + 1) * N_CORES]
        for c, g in enumerate(ranks):
            assign[c][s] = int(g)
        mx = max((int(n_act[g]) for g in ranks), default=0)
        P_list.append(max(4, _ceil_to(mx, 4)))
    offs = np.cumsum([0] + P_list[:-1]).tolist()
    NTOT = sum(P_list)
    P_dev = P_list

    # ---- host: per-core input staging ----
    wcT = np.ascontiguousarray(Wc.T)  # [h_in, o]
    wyT = np.ascontiguousarray(Wy.T)
    OB = H // 128
    bcT = np.ascontiguousarray(bc.reshape(OB, 128).T)  # [128, OB]
    byT = np.ascontiguousarray(by.reshape(OB, 128).T)

    in_maps = []
    for c in range(N_CORES):
        zT_act = np.zeros((H, NTOT), dtype=np.float32)
        madd = np.full((1, NTOT), PAD_NEG, dtype=np.float32)
        recipN = np.zeros((n_slots, 128), dtype=np.float32)
        for s in range(n_slots):
            g = assign[c][s]
            if g is None:
                continue
            n = int(n_act[g])
            if n == 0:
                continue
            o = int(offs[s])
            zT_act[:, o:o + n] = zg[g][act_idx[g]].T
            madd[0, o:o + n] = 0.0
            recipN[s, :n] = np.float32(1.0) / np.float32(n)
        in_maps.append(
            {
                "zT": zT_act,
                "wcT": wcT,
                "wyT": wyT,
                "bcT": bcT,
                "byT": byT,
                "madd": madd,
                "recipN": recipN,
            }
        )

    # ---- build + run ----
    nc = _build_program(P_dev, H)
    res = run_bass_kernel_spmd(nc, in_maps, list(range(N_CORES)))
    _last_results = res

    # ---- host: scatter back ----
    for c in range(N_CORES):
        oc = res.results[c]["out"]  # [NTOT, H]
        for s in range(n_slots):
            g = assign[c][s]
            if g is None:
                continue
            n = int(n_act[g])
            if n == 0:
                continue
            o = int(offs[s])
            out_full[g * NV + act_idx[g], :] = oc[o:o + n, :]

    return out_full
